# revision 23
# baseline (speedup 1.0000x reference)
"""Trainium2 Bass kernel for a BAN (bilinear attention network) layer.

Reference computation (per batch b, head h, hd=64, scale=hd**-0.5):
    vp = (v @ Wv + bv)  -> [V=1024, 512] split into heads [h, V, 64]
    qp = (q @ Wq + bq)  -> [Q=512, 512]  split into heads [h, Q, 64]
    logits = vp_h @ att_w_h @ qp_h^T * scale        [V, Q]
    w = softmax(logits, axis=-1)
    pooled_v = mean_v(w @ qp_h)          [64]
    pooled_q = mean_q(w^T @ vp_h)        [64]
    fused = concat per head [pooled_v, pooled_q] -> [1024]
    out = relu(fused @ Wo + bo)          [512]

Key algebraic simplifications (validated vs ref):
  * rows of w sum to 1 => pooled_q = (1/Q) * colsum_v(vp_h)
  * pooled_v = z @ (q @ Wq)_h + bq_h with z = (1/V) sum_v e[v,:]/s[v],
    e = exp(logits), s = rowsum(e); z computed as a TensorE matmul with
    the scaled reciprocal rowsums (rb, fp8) as the stationary operand
  * att_w and the 1/8 scale are folded into Wq on the host (weight-only
    transform): Wqw[d, h*64+i] = scale * sum_j Wq[d, h*64+j] att_w[h,i,j]

Performance structure (vs the 174us baseline):
  * input DMA split in 4 (core weights / b0 acts / b1 acts / tail weights)
    so the first projection starts ~3us in instead of ~20us
  * exp runs on ScalarE over [128,1024] psum PAIR tiles (two v-chunks in
    adjacent psum banks) -> 64 ACTIVATEs instead of 128, and no accum_out
    (no READ_ACCUMULATOR): softmax rowsums are computed from the fp8 e
    tiles on DVE (tensor_scalar+accum_out, 2x_2p mode) and GpSimd
    (tensor_reduce), which are otherwise idle
  * colsum z matmuls use fp8 DoubleRow perf mode (2 v-chunks per matmul,
    0.5 cycles/row) with the pair e tiles as the moving operand; all 8
    heads' z rows accumulate into ONE [8,512] psum tile (row h), drained
    by a single DVE copy (no per-head copies, no restack DMA)
  * batch-0 projection drains ride on ScalarE (idle during the prologue)
    as activation(Identity, bias); batch-1's interleave on DVE

Sharding: data-parallel over batch, 2 batches per core, params replicated,
no collectives.  Host does only layout transforms / weight folding / bf16.
"""

import numpy as np
import ml_dtypes

BF16 = ml_dtypes.bfloat16

B, V_NUM, Q_NUM = 16, 1024, 512
V_DIM, Q_DIM = 256, 128
HIDDEN, HEADS, HD = 512, 8, 64
SCALE = HD ** -0.5

N_CORES = 8
BPC = B // N_CORES          # batches per core
DC = V_DIM // 128           # d-chunks of v (2)
IB = HIDDEN // 128          # i-blocks of hidden (4)
QC = Q_NUM // 128           # q-chunks (4)
VCH = V_NUM // 128          # v-chunks of 128 (8)
VPC = VCH // 2              # v-chunk pairs (4)
NB = HIDDEN // 128          # out feature blocks (4)
KC = (2 * HEADS * HD) // 128  # fused feature chunks of 128 (8)

# packed column layout (all bf16): core weights first so compute can start
# as soon as DMA1+DMA2 land.
PCK_WCORE = 0                                  # wv (2*512) + wqw (512)
PCK_ID = PCK_WCORE + (DC + 1) * HIDDEN         # identity 8
PCK_BALL = PCK_ID + 8                          # f32 biases as bf16 bits
PCK_B0 = PCK_BALL + 2 * (2 * IB + HEADS + NB)  # per-batch vt+qt
PCK_B1 = PCK_B0 + DC * V_NUM + Q_NUM
PCK_WREST = PCK_B1 + DC * V_NUM + Q_NUM        # wq (512) + wo (8*512)
PCK_COLS = PCK_WREST + (1 + KC) * HIDDEN

_CACHE = {}


def _build_nc():
    from contextlib import ExitStack

    import concourse.bass as bass
    import concourse.tile as tile
    from concourse import bacc, mybir
    from concourse.tile import add_dep_helper

    f32 = mybir.dt.float32
    bf16 = mybir.dt.bfloat16
    fp8 = mybir.dt.float8e4
    AF = mybir.ActivationFunctionType
    ALU = mybir.AluOpType
    AX = mybir.AxisListType
    DR = mybir.MatmulPerfMode.DoubleRow

    nc = bacc.Bacc("TRN2", target_bir_lowering=False)

    packed_p = nc.declare_dram_parameter("packed", [128, PCK_COLS], bf16, isOutput=False)
    outT_p = nc.declare_dram_parameter("outT", [HIDDEN, BPC], f32, isOutput=True)

    with tile.TileContext(nc) as tc, ExitStack() as ctx:
        const = ctx.enter_context(tc.tile_pool(name="const", bufs=1))
        work = ctx.enter_context(tc.tile_pool(name="work", bufs=1))
        epool = ctx.enter_context(tc.tile_pool(name="epool", bufs=24))
        spool = ctx.enter_context(tc.tile_pool(name="spool", bufs=32))
        junk = ctx.enter_context(tc.tile_pool(name="junk", bufs=2))
        ps_pair = ctx.enter_context(tc.tile_pool(name="ps_pair", bufs=2, space="PSUM"))
        ps_proj = ctx.enter_context(tc.tile_pool(name="ps_proj", bufs=1, space="PSUM"))
        ps_z = ctx.enter_context(tc.tile_pool(name="ps_z", bufs=1, space="PSUM"))
        ps_tr = ctx.enter_context(tc.tile_pool(name="ps_tr", bufs=1, space="PSUM"))
        ps_sm = ctx.enter_context(tc.tile_pool(name="ps_sm", bufs=1, space="PSUM"))

        class SlotGuard:
            """Explicit WAR edges for psum slot reuse: the first writer of
            allocation i+bufs must wait for all readers of allocation i."""

            def __init__(self, bufs):
                self.bufs = bufs
                self.hist = []

            def alloc(self):
                self.hist.append([None, []])
                return len(self.hist) - 1

            def writer(self, idx, mi):
                if self.hist[idx][0] is None:
                    self.hist[idx][0] = mi
                    prev = idx - self.bufs
                    if prev >= 0:
                        for r in self.hist[prev][1]:
                            add_dep_helper(mi.ins, r.ins, sync=True,
                                           reason="psum slot WAR guard")
                return mi

            def reader(self, idx, mi):
                self.hist[idx][1].append(mi)
                return mi

        g_pair = SlotGuard(2)
        g_proj = SlotGuard(1)
        g_z = SlotGuard(1)
        g_tr = SlotGuard(1)
        g_sm = SlotGuard(1)

        # ---- constants + inputs: 4 DMAs over one packed param ----
        packed_sb = const.tile([128, PCK_COLS], bf16, tag="packed")
        nc.sync.dma_start(packed_sb[:, PCK_WCORE:PCK_B0],
                          packed_p[:, PCK_WCORE:PCK_B0])
        nc.sync.dma_start(packed_sb[:, PCK_B0:PCK_B1], packed_p[:, PCK_B0:PCK_B1])
        nc.sync.dma_start(packed_sb[:, PCK_B1:PCK_WREST],
                          packed_p[:, PCK_B1:PCK_WREST])
        nc.sync.dma_start(packed_sb[:, PCK_WREST:PCK_COLS],
                          packed_p[:, PCK_WREST:PCK_COLS])

        wcore_sb = packed_sb[:, PCK_WCORE:PCK_ID].rearrange(
            "p (w h) -> p w h", w=DC + 1)
        wv_sb = wcore_sb[:, 0:DC]
        wqw_sb = wcore_sb[:, DC]
        ident_sb = packed_sb[0:8, PCK_ID:PCK_ID + 8]
        ball_sb = packed_sb[:, PCK_BALL:PCK_B0].bitcast(f32)
        bv_sb = ball_sb[:, 0:IB]
        bqw_sb = ball_sb[:, IB:2 * IB]
        fb_sb = ball_sb[:, 2 * IB:2 * IB + HEADS]
        bo_sb = ball_sb[:, 2 * IB + HEADS:]
        bact_sb = [packed_sb[:, PCK_B0:PCK_B1], packed_sb[:, PCK_B1:PCK_WREST]]
        vt_sb = [a[:, :DC * V_NUM].rearrange("p (c v) -> p c v", c=DC)
                 for a in bact_sb]
        qt_sb = [a[:, DC * V_NUM:] for a in bact_sb]
        wrest_sb = packed_sb[:, PCK_WREST:PCK_COLS].rearrange(
            "p (w h) -> p w h", w=1 + KC)
        wq_sb = wrest_sb[:, 0]
        wo_sb = wrest_sb[:, 1:]

        # ---- long-lived activations ----
        vpT_sb = work.tile([128, BPC, IB, V_NUM], bf16, tag="vpt")
        qpwT_sb = work.tile([128, BPC, IB, Q_NUM], bf16, tag="qpwt")
        qp_sb = work.tile([128, BPC, QC, HIDDEN], bf16, tag="qp")
        # z rows land on psum partitions 0 (even head) / 64 (odd head); the
        # drains keep that split and two restack DMAs stack them onto
        # partitions 0-7 in parity order: row r<4 = head 2r, r>=4 = head
        # 2(r-4)+1.  pv then reads column PVCOL[h] of zT.
        zrow_sb = work.tile([128, BPC, HEADS // 2, Q_NUM], bf16, tag="zrow")
        zstack_sb = work.tile([8, BPC, Q_NUM], bf16, tag="zstack")
        zT_sb = work.tile([128, BPC, QC, HEADS], bf16, tag="zT")
        fusedT_sb = work.tile([128, KC, BPC], bf16, tag="fused")
        outT_sb = work.tile([128, NB, BPC], f32, tag="outT")
        cv_sb = work.tile([128, BPC, DC], f32, tag="cv")
        cvb_sb = work.tile([128, BPC, DC], bf16, tag="cvb")

        # softmax rowsums reduce on DVE from the bf16 e tiles: tensor_scalar
        # with accum_out runs in 4x_2p mode (~200ns/tile) where a plain
        # tensor_reduce has no fast mode; for ACT_ACCUM_PC pair-slots the
        # exp instead runs as two [128,512] ACTIVATEs with accum_out.
        ACT_ACCUM_PC = set()

        def s_reduce(e2_t, s_t, pc, b):
            for i in range(2):
                c = 2 * pc + i
                half = e2_t[:, i * Q_NUM:(i + 1) * Q_NUM]
                jt = junk.tile([128, Q_NUM], bf16, tag="junk")
                nc.vector.tensor_scalar(
                    jt[:], half, 1.0, None, ALU.mult, ALU.add,
                    accum_out=s_t[:, c:c + 1])

        # ---- prologue: projections ----
        # b0 runs with ScalarE drains (ACT idle before the first exp);
        # b1's groups are thunks spread through b0's main loop (DVE drains).
        def vpt_pair_b0(ib):
            ps = ps_pair.tile([128, 2 * Q_NUM], f32, tag="pair")
            gi = g_pair.alloc()
            for vb in range(2):
                for dc in range(DC):
                    g_pair.writer(gi, nc.tensor.matmul(
                        ps[:, vb * 512:(vb + 1) * 512],
                        lhsT=wv_sb[:, dc, ib * 128:(ib + 1) * 128],
                        rhs=vt_sb[0][:, dc, vb * 512:(vb + 1) * 512],
                        start=(dc == 0), stop=(dc == DC - 1)))
            g_pair.reader(gi, nc.scalar.activation(
                vpT_sb[:, 0, ib, :], ps[:], AF.Identity,
                bias=bv_sb[:, ib:ib + 1]))

        def qpwt_b0(ib):
            ps = ps_proj.tile([128, Q_NUM], f32, tag="proj")
            gi = g_proj.alloc()
            g_proj.writer(gi, nc.tensor.matmul(
                ps[:], lhsT=wqw_sb[:, ib * 128:(ib + 1) * 128],
                rhs=qt_sb[0][:], start=True, stop=True))
            g_proj.reader(gi, nc.scalar.activation(
                qpwT_sb[:, 0, ib, :], ps[:], AF.Identity,
                bias=bqw_sb[:, ib:ib + 1]))

        def prologue_thunks(b, skip_ib0=False):
            """Projection/pooled-q thunks (proj psum bank, DVE drains).
            skip_ib0: batch 0's ib=0 groups were emitted directly with
            ScalarE drains before the main loop."""
            thunks = []

            def th(fn):
                thunks.append(fn)

            for ib in range(1 if skip_ib0 else 0, IB):
                for vb in range(2):
                    def vpt_group(ib=ib, vb=vb):
                        ps = ps_proj.tile([128, Q_NUM], f32, tag="proj")
                        gi = g_proj.alloc()
                        for dc in range(DC):
                            g_proj.writer(gi, nc.tensor.matmul(
                                ps[:],
                                lhsT=wv_sb[:, dc, ib * 128:(ib + 1) * 128],
                                rhs=vt_sb[b][:, dc, vb * 512:(vb + 1) * 512],
                                start=(dc == 0), stop=(dc == DC - 1)))
                        g_proj.reader(gi, nc.vector.tensor_scalar_add(
                            vpT_sb[:, b, ib, vb * 512:(vb + 1) * 512],
                            ps[:], bv_sb[:, ib:ib + 1]))
                    th(vpt_group)

                def qpwt_group(ib=ib):
                    ps = ps_proj.tile([128, Q_NUM], f32, tag="proj")
                    gi = g_proj.alloc()
                    g_proj.writer(gi, nc.tensor.matmul(
                        ps[:], lhsT=wqw_sb[:, ib * 128:(ib + 1) * 128],
                        rhs=qt_sb[b][:], start=True, stop=True))
                    g_proj.reader(gi, nc.vector.tensor_scalar_add(
                        qpwT_sb[:, b, ib, :], ps[:], bqw_sb[:, ib:ib + 1]))
                th(qpwt_group)

            # qp feeds only the z tail; always thunked, DVE drains
            for qc in range(QC):
                def qp_group(qc=qc):
                    ps = ps_proj.tile([128, HIDDEN], f32, tag="proj")
                    gi = g_proj.alloc()
                    g_proj.writer(gi, nc.tensor.matmul(
                        ps[:], lhsT=qt_sb[b][:, qc * 128:(qc + 1) * 128],
                        rhs=wq_sb[:], start=True, stop=True))
                    g_proj.reader(gi, nc.vector.tensor_copy(
                        qp_sb[:, b, qc, :], ps[:]))
                th(qp_group)

            def cv_group():
                for dc in range(DC):
                    jt = junk.tile([128, V_NUM], bf16, tag="junkb")
                    nc.vector.tensor_scalar(
                        jt[:], vt_sb[b][:, dc, :], 1.0, None, ALU.mult,
                        ALU.add, accum_out=cv_sb[:, b, dc:dc + 1])
                    nc.gpsimd.tensor_copy(
                        cvb_sb[:, b, dc:dc + 1], cv_sb[:, b, dc:dc + 1])
            th(cv_group)
            for ib in range(IB):
                for half in range(2):
                    def pq_group(ib=ib, half=half):
                        h = 2 * ib + half
                        psq = ps_sm.tile([128, 8], f32, tag="sm")
                        gi = g_sm.alloc()
                        for dc in range(DC):
                            g_sm.writer(gi, nc.tensor.matmul(
                                psq[64:128, 0:1],
                                lhsT=wv_sb[:, dc, ib * 128 + 64 * half: ib * 128 + 64 * half + 64],
                                rhs=cvb_sb[:, b, dc:dc + 1],
                                start=(dc == 0), stop=(dc == DC - 1)))
                        g_sm.reader(gi, nc.vector.tensor_scalar(
                            fusedT_sb[64:128, h, b:b+1], psq[64:128, 0:1],
                            1.0 / Q_NUM, fb_sb[64:128, h:h + 1],
                            ALU.mult, ALU.add))
                    th(pq_group)
            return thunks

        # ---- z tail: restack z rows, transpose, pooled_v ----
        PVCOL = [h // 2 if h % 2 == 0 else 4 + h // 2 for h in range(HEADS)]

        def ztail_thunks(b):
            thunks = []

            def th(fn):
                thunks.append(fn)

            def restack():
                nc.sync.dma_start(
                    zstack_sb[0:4, b, :],
                    zrow_sb[0:1, b].rearrange("p t q -> p (t q)"))
                nc.sync.dma_start(
                    zstack_sb[4:8, b, :],
                    zrow_sb[64:65, b].rearrange("p t q -> p (t q)"))
            th(restack)
            for qc in range(QC):
                def tr_group(qc=qc):
                    pst = ps_tr.tile([128, 8], bf16, tag="tr")
                    gi = g_tr.alloc()
                    g_tr.writer(gi, nc.tensor.transpose(
                        pst[:], zstack_sb[:, b, qc * 128:(qc + 1) * 128],
                        ident_sb[:]))
                    g_tr.reader(gi, nc.vector.tensor_copy(
                        zT_sb[:, b, qc, :], pst[:]))
                th(tr_group)
            for h in range(HEADS):
                def pv_group(h=h):
                    psv = ps_sm.tile([128, 8], f32, tag="sm")
                    gi = g_sm.alloc()
                    for qc in range(QC):
                        g_sm.writer(gi, nc.tensor.matmul(
                            psv[0:64, 0:1],
                            lhsT=qp_sb[:, b, qc, h * 64:(h + 1) * 64],
                            rhs=zT_sb[:, b, qc, PVCOL[h]:PVCOL[h] + 1],
                            start=(qc == 0), stop=(qc == QC - 1)))
                    if b == BPC - 1:
                        # last batch's tail: ACT is idle, keep DVE free
                        g_sm.reader(gi, nc.scalar.activation(
                            fusedT_sb[0:64, h, b:b+1], psv[0:64, 0:1],
                            AF.Identity, bias=fb_sb[0:64, h:h + 1]))
                    else:
                        g_sm.reader(gi, nc.vector.tensor_scalar(
                            fusedT_sb[0:64, h, b:b+1], psv[0:64, 0:1],
                            1.0, fb_sb[0:64, h:h + 1],
                            ALU.mult, ALU.add))
                th(pv_group)
            return thunks

        # ---- main loop ----
        # Per head-pair t: 8 pair-tiles (2 sides x 4 pair-chunks), each =
        # 2 logits matmuls -> one [128,1024] exp -> fp8 e2 tile; rowsums on
        # DVE/GpSimd.  Pair t-1's DoubleRow colsums + prologue thunks fill
        # the PE slack between pair-tile matmuls.
        def emit_main(b, pre_work):
            pending = None

            def colsum(pend, pc):
                t, zps, zgi, psides = pend
                for h, e_list, rb_t in psides:
                    hb = 64 * (h % 2)
                    for i in range(2):
                        c = 2 * pc + i
                        g_z.writer(zgi, nc.tensor.matmul(
                            zps[hb:hb + 1, :],
                            lhsT=rb_t[:, c:c + 1],
                            rhs=e_list[pc][:, i * Q_NUM:(i + 1) * Q_NUM],
                            start=(c == 0), stop=(c == VCH - 1)))
                if pc == VPC - 1:
                    for h, e_list, rb_t in psides:
                        hb = 64 * (h % 2)
                        g_z.reader(zgi, nc.vector.tensor_copy(
                            zrow_sb[hb:hb + 1, b, t, :], zps[hb:hb + 1, :]))

            for t in range(HEADS // 2):
                sides = []
                for h in (2 * t, 2 * t + 1):
                    s_t = spool.tile([128, VCH], f32, tag="s")
                    sides.append([h, 64 * (h % 2), s_t, []])
                for pc in range(VPC):
                    for side in sides:
                        h, hb, s_t, e_list = side
                        ps = ps_pair.tile([128, 2 * Q_NUM], f32, tag="pair")
                        gi = g_pair.alloc()
                        for i in range(2):
                            c = 2 * pc + i
                            g_pair.writer(gi, nc.tensor.matmul(
                                ps[:, i * Q_NUM:(i + 1) * Q_NUM],
                                lhsT=vpT_sb[hb:hb + 64, b, t, c * 128:(c + 1) * 128],
                                rhs=qpwT_sb[hb:hb + 64, b, t, :],
                                start=True, stop=True))
                        e2_t = epool.tile([128, 2 * Q_NUM], bf16, tag="e")
                        if (t, pc) in ACT_ACCUM_PC:
                            for i in range(2):
                                c = 2 * pc + i
                                g_pair.reader(gi, nc.scalar.activation(
                                    e2_t[:, i * Q_NUM:(i + 1) * Q_NUM],
                                    ps[:, i * Q_NUM:(i + 1) * Q_NUM], AF.Exp,
                                    accum_out=s_t[:, c:c + 1]))
                        else:
                            g_pair.reader(gi, nc.scalar.activation(
                                e2_t[:], ps[:], AF.Exp))
                            s_reduce(e2_t, s_t, pc, b)
                        e_list.append(e2_t)
                    if pending is not None:
                        colsum(pending, pc)
                    for _ in range(2):
                        if pre_work:
                            pre_work.pop(0)()
                # reciprocals for this pair (DVE, overlaps next pair)
                zps = ps_z.tile([128, Q_NUM], f32, tag="z")
                zgi = g_z.alloc()
                new_sides = []
                for h, hb, s_t, e_list in sides:
                    r_t = spool.tile([128, VCH], f32, tag="r")
                    nc.vector.reciprocal(r_t[:], s_t[:])
                    rb_t = spool.tile([128, VCH], bf16, tag="rb")
                    nc.vector.tensor_scalar_mul(
                        rb_t[:], r_t[:], 1.0 / V_NUM)
                    new_sides.append((h, e_list, rb_t))
                pending = (t, zps, zgi, new_sides)
            # last pair's colsum (trailing, overlaps next batch's stream)
            for pc in range(VPC):
                colsum(pending, pc)
                if pre_work:
                    pre_work.pop(0)()
            while pre_work:
                pre_work.pop(0)()

        vpt_pair_b0(0)
        qpwt_b0(0)
        pro0 = prologue_thunks(0, skip_ib0=True)
        emit_main(0, pro0 + prologue_thunks(1))
        emit_main(1, ztail_thunks(0))
        for fn in ztail_thunks(1):
            fn()

        # ---- epilogue: out = relu(fused @ Wo + bo), computed transposed ----
        for nb in range(NB):
            pso = ps_sm.tile([128, 8], f32, tag="sm")
            gi = g_sm.alloc()
            for kc in range(KC):
                g_sm.writer(gi, nc.tensor.matmul(
                    pso[:, 0:BPC],
                    lhsT=wo_sb[:, kc, nb * 128:(nb + 1) * 128],
                    rhs=fusedT_sb[:, kc, :],
                    start=(kc == 0), stop=(kc == KC - 1)))
            g_sm.reader(gi, nc.scalar.activation(
                outT_sb[:, nb, :], pso[:, 0:BPC], AF.Relu,
                bias=bo_sb[:, nb:nb + 1]))
        nc.sync.dma_start(
            outT_p[:].rearrange("(o p) b -> p o b", p=128), outT_sb[:])

    nc.compile()
    return nc


def _get_nc():
    if "nc" not in _CACHE:
        _CACHE["nc"] = _build_nc()
    return _CACHE["nc"]


def _host_prep(v, q, Wv, bv, Wq, bq, att_w, Wo, bo):
    """Host-side layout transforms + weight folding. Returns per-core in_maps."""
    v = np.asarray(v, np.float32)
    q = np.asarray(q, np.float32)
    Wv = np.asarray(Wv, np.float32)
    bv = np.asarray(bv, np.float32)
    Wq = np.asarray(Wq, np.float32)
    bq = np.asarray(bq, np.float32)
    att_w = np.asarray(att_w, np.float32)
    Wo = np.asarray(Wo, np.float32)
    bo = np.asarray(bo, np.float32)

    # fold att_w and softmax scale into the q projection
    Wq_h = Wq.reshape(Q_DIM, HEADS, HD)
    Wqw = (SCALE * np.einsum("dhj,hij->dhi", Wq_h, att_w)).reshape(Q_DIM, HIDDEN)
    bqw = (SCALE * np.einsum("hj,hij->hi", bq.reshape(HEADS, HD), att_w)).reshape(HIDDEN)

    wcore = np.concatenate([
        Wv.reshape(DC, 128, HIDDEN).transpose(1, 0, 2),
        Wqw.reshape(1, 128, HIDDEN).transpose(1, 0, 2),
    ], axis=1).reshape(128, (DC + 1) * HIDDEN)
    wrest = np.concatenate([
        Wq.reshape(1, 128, HIDDEN).transpose(1, 0, 2),
        Wo.reshape(KC, 128, HIDDEN).transpose(1, 0, 2),
    ], axis=1).reshape(128, (1 + KC) * HIDDEN)
    fbias = np.concatenate(
        [bq.reshape(HEADS, HD).T,
         (V_NUM / Q_NUM) * bv.reshape(HEADS, HD).T], axis=0)
    ball = np.concatenate([
        bv.reshape(IB, 128).T, bqw.reshape(IB, 128).T,
        fbias, bo.reshape(NB, 128).T], axis=1).astype(np.float32)
    ident = np.zeros((128, 8), np.float32)
    ident[:8, :8] = np.eye(8)
    head_cols = np.concatenate([
        wcore.astype(BF16), ident.astype(BF16),
        np.ascontiguousarray(ball).view(BF16)], axis=1)
    wrest_cols = wrest.astype(BF16)
    in_maps = []
    for i in range(N_CORES):
        bcols = []
        for bi in range(BPC):
            bidx = i * BPC + bi
            vt = v[bidx].T.reshape(DC, 128, V_NUM).transpose(1, 0, 2).reshape(128, DC * V_NUM)
            qt = q[bidx].T
            bcols.append(np.concatenate([vt.astype(BF16), qt.astype(BF16)], axis=1))
        packed = np.concatenate([head_cols] + bcols + [wrest_cols], axis=1)
        in_maps.append({"packed": np.ascontiguousarray(packed)})
    return in_maps


def kernel(**inputs):
    from concourse.bass_utils import run_bass_kernel_spmd

    nc = _get_nc()
    in_maps = _host_prep(**inputs)
    res = run_bass_kernel_spmd(nc, in_maps, core_ids=list(range(N_CORES)))
    out = np.empty((B, HIDDEN), np.float32)
    for i in range(N_CORES):
        out[i * BPC:(i + 1) * BPC] = np.asarray(res.results[i]["outT"]).T
    return out


# revision 30
# speedup vs baseline: 1.0421x; 1.0421x over previous
"""Trainium2 Bass kernel for a BAN (bilinear attention network) layer.

Reference computation (per batch b, head h, hd=64, scale=hd**-0.5):
    vp = (v @ Wv + bv)  -> [V=1024, 512] split into heads [h, V, 64]
    qp = (q @ Wq + bq)  -> [Q=512, 512]  split into heads [h, Q, 64]
    logits = vp_h @ att_w_h @ qp_h^T * scale        [V, Q]
    w = softmax(logits, axis=-1)
    pooled_v = mean_v(w @ qp_h)          [64]
    pooled_q = mean_q(w^T @ vp_h)        [64]
    fused = concat per head [pooled_v, pooled_q] -> [1024]
    out = relu(fused @ Wo + bo)          [512]

Key algebraic simplifications (validated vs ref):
  * rows of w sum to 1 => pooled_q = (1/Q) * colsum_v(vp_h)
  * pooled_v = z @ (q @ Wq)_h + bq_h with z = (1/V) sum_v e[v,:]/s[v],
    e = exp(logits), s = rowsum(e); z computed as a TensorE matmul with
    the scaled reciprocal rowsums (rb, fp8) as the stationary operand
  * att_w and the 1/8 scale are folded into Wq on the host (weight-only
    transform): Wqw[d, h*64+i] = scale * sum_j Wq[d, h*64+j] att_w[h,i,j]

Performance structure (vs the 174us baseline):
  * input DMA split in 4 (core weights / b0 acts / b1 acts / tail weights)
    so the first projection starts ~3us in instead of ~20us
  * exp runs on ScalarE over [128,1024] psum PAIR tiles (two v-chunks in
    adjacent psum banks) -> 64 ACTIVATEs instead of 128, and no accum_out
    (no READ_ACCUMULATOR): softmax rowsums are computed from the fp8 e
    tiles on DVE (tensor_scalar+accum_out, 2x_2p mode) and GpSimd
    (tensor_reduce), which are otherwise idle
  * colsum z matmuls use fp8 DoubleRow perf mode (2 v-chunks per matmul,
    0.5 cycles/row) with the pair e tiles as the moving operand; all 8
    heads' z rows accumulate into ONE [8,512] psum tile (row h), drained
    by a single DVE copy (no per-head copies, no restack DMA)
  * batch-0 projection drains ride on ScalarE (idle during the prologue)
    as activation(Identity, bias); batch-1's interleave on DVE

Sharding: data-parallel over batch, 2 batches per core, params replicated,
no collectives.  Host does only layout transforms / weight folding / bf16.
"""

import numpy as np
import ml_dtypes

BF16 = ml_dtypes.bfloat16

B, V_NUM, Q_NUM = 16, 1024, 512
V_DIM, Q_DIM = 256, 128
HIDDEN, HEADS, HD = 512, 8, 64
SCALE = HD ** -0.5

N_CORES = 8
BPC = B // N_CORES          # batches per core
DC = V_DIM // 128           # d-chunks of v (2)
IB = HIDDEN // 128          # i-blocks of hidden (4)
QC = Q_NUM // 128           # q-chunks (4)
VCH = V_NUM // 128          # v-chunks of 128 (8)
VPC = VCH // 2              # v-chunk pairs (4)
NB = HIDDEN // 128          # out feature blocks (4)
KC = (2 * HEADS * HD) // 128  # fused feature chunks of 128 (8)

# packed column layout (all bf16): core weights first so compute can start
# as soon as DMA1+DMA2 land.
PCK_WCORE = 0                                  # wv (2*512) + wqw (512)
PCK_ID = PCK_WCORE + (DC + 1) * HIDDEN         # identity 8
PCK_BALL = PCK_ID + 8                          # f32 biases as bf16 bits
PCK_B0 = PCK_BALL + 2 * (2 * IB + HEADS + NB)  # per-batch vt+qt
PCK_B1 = PCK_B0 + DC * V_NUM + Q_NUM
PCK_WREST = PCK_B1 + DC * V_NUM + Q_NUM        # wq (512) + wo (8*512)
PCK_COLS = PCK_WREST + (1 + KC) * HIDDEN

_CACHE = {}


def _build_nc():
    from contextlib import ExitStack

    import concourse.bass as bass
    import concourse.tile as tile
    from concourse import bacc, mybir
    from concourse.tile import add_dep_helper

    f32 = mybir.dt.float32
    bf16 = mybir.dt.bfloat16
    fp8 = mybir.dt.float8e4
    AF = mybir.ActivationFunctionType
    ALU = mybir.AluOpType
    AX = mybir.AxisListType
    DR = mybir.MatmulPerfMode.DoubleRow

    nc = bacc.Bacc("TRN2", target_bir_lowering=False)

    packed_p = nc.declare_dram_parameter("packed", [128, PCK_COLS], bf16, isOutput=False)
    outT_p = nc.declare_dram_parameter("outT", [HIDDEN, BPC], f32, isOutput=True)

    with tile.TileContext(nc) as tc, ExitStack() as ctx:
        const = ctx.enter_context(tc.tile_pool(name="const", bufs=1))
        work = ctx.enter_context(tc.tile_pool(name="work", bufs=1))
        epool = ctx.enter_context(tc.tile_pool(name="epool", bufs=40))
        spool = ctx.enter_context(tc.tile_pool(name="spool", bufs=32))
        junk = ctx.enter_context(tc.tile_pool(name="junk", bufs=2))
        ps_pair = ctx.enter_context(tc.tile_pool(name="ps_pair", bufs=2, space="PSUM"))
        ps_proj = ctx.enter_context(tc.tile_pool(name="ps_proj", bufs=1, space="PSUM"))
        ps_z = ctx.enter_context(tc.tile_pool(name="ps_z", bufs=1, space="PSUM"))
        ps_tr = ctx.enter_context(tc.tile_pool(name="ps_tr", bufs=1, space="PSUM"))
        ps_sm = ctx.enter_context(tc.tile_pool(name="ps_sm", bufs=1, space="PSUM"))

        class SlotGuard:
            """Explicit WAR edges for psum slot reuse: the first writer of
            allocation i+bufs must wait for all readers of allocation i."""

            def __init__(self, bufs):
                self.bufs = bufs
                self.hist = []

            def alloc(self):
                self.hist.append([None, []])
                return len(self.hist) - 1

            def writer(self, idx, mi):
                if self.hist[idx][0] is None:
                    self.hist[idx][0] = mi
                    prev = idx - self.bufs
                    if prev >= 0:
                        for r in self.hist[prev][1]:
                            add_dep_helper(mi.ins, r.ins, sync=True,
                                           reason="psum slot WAR guard")
                return mi

            def reader(self, idx, mi):
                self.hist[idx][1].append(mi)
                return mi

        g_pair = SlotGuard(2)
        g_proj = SlotGuard(1)
        g_z = SlotGuard(1)
        g_tr = SlotGuard(1)
        g_sm = SlotGuard(1)

        # prewarm the ACT spline table (exp_and_others) while the input DMAs
        # are in flight, so the 1.3us ACT_TABLE_LOAD is off the critical path
        warm_sb = const.tile([1, 8], f32, tag="warm")
        nc.vector.memset(warm_sb[:], 0.0)
        nc.scalar.activation(warm_sb[0:1, 4:5], warm_sb[0:1, 0:1], AF.Exp)

        # ---- constants + inputs: 4 DMAs over one packed param ----
        packed_sb = const.tile([128, PCK_COLS], bf16, tag="packed")
        nc.sync.dma_start(packed_sb[:, PCK_WCORE:PCK_B0],
                          packed_p[:, PCK_WCORE:PCK_B0])
        nc.sync.dma_start(packed_sb[:, PCK_B0:PCK_B1], packed_p[:, PCK_B0:PCK_B1])
        nc.sync.dma_start(packed_sb[:, PCK_B1:PCK_WREST],
                          packed_p[:, PCK_B1:PCK_WREST])
        nc.sync.dma_start(packed_sb[:, PCK_WREST:PCK_COLS],
                          packed_p[:, PCK_WREST:PCK_COLS])

        wcore_sb = packed_sb[:, PCK_WCORE:PCK_ID].rearrange(
            "p (w h) -> p w h", w=DC + 1)
        wv_sb = wcore_sb[:, 0:DC]
        wqw_sb = wcore_sb[:, DC]
        ident_sb = packed_sb[0:8, PCK_ID:PCK_ID + 8]
        ball_sb = packed_sb[:, PCK_BALL:PCK_B0].bitcast(f32)
        bv_sb = ball_sb[:, 0:IB]
        bqw_sb = ball_sb[:, IB:2 * IB]
        fb_sb = ball_sb[:, 2 * IB:2 * IB + HEADS]
        bo_sb = ball_sb[:, 2 * IB + HEADS:]
        bact_sb = [packed_sb[:, PCK_B0:PCK_B1], packed_sb[:, PCK_B1:PCK_WREST]]
        vt_sb = [a[:, :DC * V_NUM].rearrange("p (c v) -> p c v", c=DC)
                 for a in bact_sb]
        qt_sb = [a[:, DC * V_NUM:] for a in bact_sb]
        wrest_sb = packed_sb[:, PCK_WREST:PCK_COLS].rearrange(
            "p (w h) -> p w h", w=1 + KC)
        wq_sb = wrest_sb[:, 0]
        wo_sb = wrest_sb[:, 1:]

        # ---- long-lived activations ----
        vpT_sb = work.tile([128, BPC, IB, V_NUM], bf16, tag="vpt")
        qpwT_sb = work.tile([128, BPC, IB, Q_NUM], bf16, tag="qpwt")
        qp_sb = work.tile([128, BPC, QC, HIDDEN], bf16, tag="qp")
        # z rows land on psum partitions 0 (even head) / 64 (odd head); the
        # drains keep that split and two restack DMAs stack them onto
        # partitions 0-7 in parity order: row r<4 = head 2r, r>=4 = head
        # 2(r-4)+1.  pv then reads column PVCOL[h] of zT.
        zrow_sb = work.tile([128, BPC, HEADS // 2, Q_NUM], bf16, tag="zrow")
        zstack_sb = work.tile([8, BPC, Q_NUM], bf16, tag="zstack")
        zT_sb = work.tile([128, BPC, QC, HEADS], bf16, tag="zT")
        fusedT_sb = work.tile([128, KC, BPC], bf16, tag="fused")
        outT_sb = work.tile([128, NB, BPC], f32, tag="outT")
        cv_sb = work.tile([128, BPC, DC], f32, tag="cv")
        cvb_sb = work.tile([128, BPC, DC], bf16, tag="cvb")

        # softmax rowsums: every engine reduces at ~1 elem/cycle (the DVE
        # CACHE_REDUCE path gets no packing modes, HW-measured 677ns/tile,
        # and GpSimd's ISA rejects TensorScalarPtr), so the second pass over
        # e is split between DVE (tensor_scalar+accum_out, 759ns/tile) and
        # ACT (exp as two [128,512] ACTIVATEs with accum_out, +711ns/pair
        # over a fused pair-exp) to balance the two engines.
        ACT_ACCUM_PC = {(0, 1), (0, 3), (1, 1), (2, 1), (2, 3), (3, 1)}

        def s_reduce(e2_t, s_t, pc, b):
            for i in range(2):
                c = 2 * pc + i
                half = e2_t[:, i * Q_NUM:(i + 1) * Q_NUM]
                jt = junk.tile([128, Q_NUM], bf16, tag="junk")
                nc.vector.tensor_scalar(
                    jt[:], half, 1.0, None, ALU.mult, ALU.add,
                    accum_out=s_t[:, c:c + 1])

        # ---- prologue: projections ----
        # b0 runs with ScalarE drains (ACT idle before the first exp);
        # b1's groups are thunks spread through b0's main loop (DVE drains).
        def vpt_pair_b0(ib):
            ps = ps_pair.tile([128, 2 * Q_NUM], f32, tag="pair")
            gi = g_pair.alloc()
            for vb in range(2):
                for dc in range(DC):
                    g_pair.writer(gi, nc.tensor.matmul(
                        ps[:, vb * 512:(vb + 1) * 512],
                        lhsT=wv_sb[:, dc, ib * 128:(ib + 1) * 128],
                        rhs=vt_sb[0][:, dc, vb * 512:(vb + 1) * 512],
                        start=(dc == 0), stop=(dc == DC - 1)))
            g_pair.reader(gi, nc.scalar.activation(
                vpT_sb[:, 0, ib, :], ps[:], AF.Identity,
                bias=bv_sb[:, ib:ib + 1]))

        def qpwt_b0(ib):
            ps = ps_proj.tile([128, Q_NUM], f32, tag="proj")
            gi = g_proj.alloc()
            g_proj.writer(gi, nc.tensor.matmul(
                ps[:], lhsT=wqw_sb[:, ib * 128:(ib + 1) * 128],
                rhs=qt_sb[0][:], start=True, stop=True))
            g_proj.reader(gi, nc.scalar.activation(
                qpwT_sb[:, 0, ib, :], ps[:], AF.Identity,
                bias=bqw_sb[:, ib:ib + 1]))

        def qp_group(b, qc, act):
            ps = ps_proj.tile([128, HIDDEN], f32, tag="proj")
            gi = g_proj.alloc()
            g_proj.writer(gi, nc.tensor.matmul(
                ps[:], lhsT=qt_sb[b][:, qc * 128:(qc + 1) * 128],
                rhs=wq_sb[:], start=True, stop=True))
            if act:
                g_proj.reader(gi, nc.scalar.activation(
                    qp_sb[:, b, qc, :], ps[:], AF.Copy))
            else:
                g_proj.reader(gi, nc.vector.tensor_copy(
                    qp_sb[:, b, qc, :], ps[:]))

        def prologue_thunks(b, skip_ib0=False):
            """Projection/pooled-q thunks (proj psum bank, DVE drains).
            skip_ib0: batch 0's ib=0 groups were emitted directly with
            ScalarE drains before the main loop."""
            thunks = []

            def th(fn):
                thunks.append(fn)

            for ib in range(1 if skip_ib0 else 0, IB):
                for vb in range(2):
                    def vpt_group(ib=ib, vb=vb):
                        ps = ps_proj.tile([128, Q_NUM], f32, tag="proj")
                        gi = g_proj.alloc()
                        for dc in range(DC):
                            g_proj.writer(gi, nc.tensor.matmul(
                                ps[:],
                                lhsT=wv_sb[:, dc, ib * 128:(ib + 1) * 128],
                                rhs=vt_sb[b][:, dc, vb * 512:(vb + 1) * 512],
                                start=(dc == 0), stop=(dc == DC - 1)))
                        g_proj.reader(gi, nc.vector.tensor_scalar_add(
                            vpT_sb[:, b, ib, vb * 512:(vb + 1) * 512],
                            ps[:], bv_sb[:, ib:ib + 1]))
                    th(vpt_group)

                def qpwt_group(ib=ib):
                    ps = ps_proj.tile([128, Q_NUM], f32, tag="proj")
                    gi = g_proj.alloc()
                    g_proj.writer(gi, nc.tensor.matmul(
                        ps[:], lhsT=wqw_sb[:, ib * 128:(ib + 1) * 128],
                        rhs=qt_sb[b][:], start=True, stop=True))
                    g_proj.reader(gi, nc.vector.tensor_scalar_add(
                        qpwT_sb[:, b, ib, :], ps[:], bqw_sb[:, ib:ib + 1]))
                th(qpwt_group)

            # qp feeds only the z tail; b1's groups run at its tail with ACT
            # drains (ACT idle there), b0's are thunked here with DVE drains
            if b == 0:
                for qc in range(QC):
                    th(lambda qc=qc: qp_group(b, qc, act=False))

            def cv_group():
                for dc in range(DC):
                    jt = junk.tile([128, V_NUM], bf16, tag="junkb")
                    nc.vector.tensor_scalar(
                        jt[:], vt_sb[b][:, dc, :], 1.0, None, ALU.mult,
                        ALU.add, accum_out=cv_sb[:, b, dc:dc + 1])
                    nc.gpsimd.tensor_copy(
                        cvb_sb[:, b, dc:dc + 1], cv_sb[:, b, dc:dc + 1])
            th(cv_group)
            for ib in range(IB):
                for half in range(2):
                    def pq_group(ib=ib, half=half):
                        h = 2 * ib + half
                        psq = ps_sm.tile([128, 8], f32, tag="sm")
                        gi = g_sm.alloc()
                        for dc in range(DC):
                            g_sm.writer(gi, nc.tensor.matmul(
                                psq[64:128, 0:1],
                                lhsT=wv_sb[:, dc, ib * 128 + 64 * half: ib * 128 + 64 * half + 64],
                                rhs=cvb_sb[:, b, dc:dc + 1],
                                start=(dc == 0), stop=(dc == DC - 1)))
                        g_sm.reader(gi, nc.vector.tensor_scalar(
                            fusedT_sb[64:128, h, b:b+1], psq[64:128, 0:1],
                            1.0 / Q_NUM, fb_sb[64:128, h:h + 1],
                            ALU.mult, ALU.add))
                    th(pq_group)
            return thunks

        # ---- z tail: restack z rows, transpose, pooled_v ----
        PVCOL = [h // 2 if h % 2 == 0 else 4 + h // 2 for h in range(HEADS)]

        def ztail_thunks(b):
            thunks = []

            def th(fn):
                thunks.append(fn)

            if b == BPC - 1:
                for qc in range(QC):
                    th(lambda qc=qc: qp_group(b, qc, act=True))

            def restack():
                nc.sync.dma_start(
                    zstack_sb[0:4, b, :],
                    zrow_sb[0:1, b].rearrange("p t q -> p (t q)"))
                nc.sync.dma_start(
                    zstack_sb[4:8, b, :],
                    zrow_sb[64:65, b].rearrange("p t q -> p (t q)"))
            th(restack)
            for qc in range(QC):
                def tr_group(qc=qc):
                    pst = ps_tr.tile([128, 8], bf16, tag="tr")
                    gi = g_tr.alloc()
                    g_tr.writer(gi, nc.tensor.transpose(
                        pst[:], zstack_sb[:, b, qc * 128:(qc + 1) * 128],
                        ident_sb[:]))
                    g_tr.reader(gi, nc.vector.tensor_copy(
                        zT_sb[:, b, qc, :], pst[:]))
                th(tr_group)
            for h in range(HEADS):
                def pv_group(h=h):
                    psv = ps_sm.tile([128, 8], f32, tag="sm")
                    gi = g_sm.alloc()
                    for qc in range(QC):
                        g_sm.writer(gi, nc.tensor.matmul(
                            psv[0:64, 0:1],
                            lhsT=qp_sb[:, b, qc, h * 64:(h + 1) * 64],
                            rhs=zT_sb[:, b, qc, PVCOL[h]:PVCOL[h] + 1],
                            start=(qc == 0), stop=(qc == QC - 1)))
                    if b == BPC - 1:
                        # last batch's tail: ACT is idle, keep DVE free
                        g_sm.reader(gi, nc.scalar.activation(
                            fusedT_sb[0:64, h, b:b+1], psv[0:64, 0:1],
                            AF.Identity, bias=fb_sb[0:64, h:h + 1]))
                    else:
                        g_sm.reader(gi, nc.vector.tensor_scalar(
                            fusedT_sb[0:64, h, b:b+1], psv[0:64, 0:1],
                            1.0, fb_sb[0:64, h:h + 1],
                            ALU.mult, ALU.add))
                th(pv_group)
            return thunks

        # ---- main loop ----
        # Per head-pair t: 8 pair-tiles (2 sides x 4 pair-chunks), each =
        # 2 logits matmuls -> one [128,1024] exp -> fp8 e2 tile; rowsums on
        # DVE/GpSimd.  Pair t-1's DoubleRow colsums + prologue thunks fill
        # the PE slack between pair-tile matmuls.
        def emit_main(b, pre_work):
            pending = None

            def colsum(pend, pc):
                t, zps, zgi, psides = pend
                for h, e_list, rb_t in psides:
                    hb = 64 * (h % 2)
                    for i in range(2):
                        c = 2 * pc + i
                        g_z.writer(zgi, nc.tensor.matmul(
                            zps[hb:hb + 1, :],
                            lhsT=rb_t[:, c:c + 1],
                            rhs=e_list[pc][:, i * Q_NUM:(i + 1) * Q_NUM],
                            start=(c == 0), stop=(c == VCH - 1)))
                if pc == VPC - 1:
                    for h, e_list, rb_t in psides:
                        hb = 64 * (h % 2)
                        if b == BPC - 1 and t == HEADS // 2 - 1:
                            # after the final exp: ACT is idle
                            g_z.reader(zgi, nc.scalar.activation(
                                zrow_sb[hb:hb + 1, b, t, :],
                                zps[hb:hb + 1, :], AF.Copy))
                        else:
                            g_z.reader(zgi, nc.vector.tensor_copy(
                                zrow_sb[hb:hb + 1, b, t, :], zps[hb:hb + 1, :]))

            for t in range(HEADS // 2):
                sides = []
                for h in (2 * t, 2 * t + 1):
                    s_t = spool.tile([128, VCH], f32, tag="s")
                    sides.append([h, 64 * (h % 2), s_t, []])
                for pc in range(VPC):
                    for side in sides:
                        h, hb, s_t, e_list = side
                        ps = ps_pair.tile([128, 2 * Q_NUM], f32, tag="pair")
                        gi = g_pair.alloc()
                        for i in range(2):
                            c = 2 * pc + i
                            g_pair.writer(gi, nc.tensor.matmul(
                                ps[:, i * Q_NUM:(i + 1) * Q_NUM],
                                lhsT=vpT_sb[hb:hb + 64, b, t, c * 128:(c + 1) * 128],
                                rhs=qpwT_sb[hb:hb + 64, b, t, :],
                                start=True, stop=True))
                        e2_t = epool.tile([128, 2 * Q_NUM], bf16, tag="e")
                        if (t, pc) in ACT_ACCUM_PC:
                            for i in range(2):
                                c = 2 * pc + i
                                g_pair.reader(gi, nc.scalar.activation(
                                    e2_t[:, i * Q_NUM:(i + 1) * Q_NUM],
                                    ps[:, i * Q_NUM:(i + 1) * Q_NUM], AF.Exp,
                                    accum_out=s_t[:, c:c + 1]))
                        else:
                            g_pair.reader(gi, nc.scalar.activation(
                                e2_t[:], ps[:], AF.Exp))
                            s_reduce(e2_t, s_t, pc, b)
                        e_list.append(e2_t)
                    if pending is not None:
                        colsum(pending, pc)
                    for _ in range(2):
                        if pre_work:
                            pre_work.pop(0)()
                # reciprocals for this pair (DVE, overlaps next pair)
                zps = ps_z.tile([128, Q_NUM], f32, tag="z")
                zgi = g_z.alloc()
                new_sides = []
                for h, hb, s_t, e_list in sides:
                    r_t = spool.tile([128, VCH], f32, tag="r")
                    nc.vector.reciprocal(r_t[:], s_t[:])
                    rb_t = spool.tile([128, VCH], bf16, tag="rb")
                    nc.vector.tensor_scalar_mul(
                        rb_t[:], r_t[:], 1.0 / V_NUM)
                    new_sides.append((h, e_list, rb_t))
                pending = (t, zps, zgi, new_sides)
            # last pair's colsum (trailing, overlaps next batch's stream)
            for pc in range(VPC):
                colsum(pending, pc)
                if pre_work:
                    pre_work.pop(0)()
            while pre_work:
                pre_work.pop(0)()

        vpt_pair_b0(0)
        qpwt_b0(0)
        pro0 = prologue_thunks(0, skip_ib0=True)
        emit_main(0, pro0 + prologue_thunks(1))
        emit_main(1, ztail_thunks(0))
        for fn in ztail_thunks(1):
            fn()

        # ---- epilogue: out = relu(fused @ Wo + bo), computed transposed ----
        for nb in range(NB):
            pso = ps_sm.tile([128, 8], f32, tag="sm")
            gi = g_sm.alloc()
            for kc in range(KC):
                g_sm.writer(gi, nc.tensor.matmul(
                    pso[:, 0:BPC],
                    lhsT=wo_sb[:, kc, nb * 128:(nb + 1) * 128],
                    rhs=fusedT_sb[:, kc, :],
                    start=(kc == 0), stop=(kc == KC - 1)))
            g_sm.reader(gi, nc.scalar.activation(
                outT_sb[:, nb, :], pso[:, 0:BPC], AF.Relu,
                bias=bo_sb[:, nb:nb + 1]))
        nc.sync.dma_start(
            outT_p[:].rearrange("(o p) b -> p o b", p=128), outT_sb[:])

    nc.compile()
    return nc


def _get_nc():
    if "nc" not in _CACHE:
        _CACHE["nc"] = _build_nc()
    return _CACHE["nc"]


def _host_prep(v, q, Wv, bv, Wq, bq, att_w, Wo, bo):
    """Host-side layout transforms + weight folding. Returns per-core in_maps."""
    v = np.asarray(v, np.float32)
    q = np.asarray(q, np.float32)
    Wv = np.asarray(Wv, np.float32)
    bv = np.asarray(bv, np.float32)
    Wq = np.asarray(Wq, np.float32)
    bq = np.asarray(bq, np.float32)
    att_w = np.asarray(att_w, np.float32)
    Wo = np.asarray(Wo, np.float32)
    bo = np.asarray(bo, np.float32)

    # fold att_w and softmax scale into the q projection
    Wq_h = Wq.reshape(Q_DIM, HEADS, HD)
    Wqw = (SCALE * np.einsum("dhj,hij->dhi", Wq_h, att_w)).reshape(Q_DIM, HIDDEN)
    bqw = (SCALE * np.einsum("hj,hij->hi", bq.reshape(HEADS, HD), att_w)).reshape(HIDDEN)

    wcore = np.concatenate([
        Wv.reshape(DC, 128, HIDDEN).transpose(1, 0, 2),
        Wqw.reshape(1, 128, HIDDEN).transpose(1, 0, 2),
    ], axis=1).reshape(128, (DC + 1) * HIDDEN)
    wrest = np.concatenate([
        Wq.reshape(1, 128, HIDDEN).transpose(1, 0, 2),
        Wo.reshape(KC, 128, HIDDEN).transpose(1, 0, 2),
    ], axis=1).reshape(128, (1 + KC) * HIDDEN)
    fbias = np.concatenate(
        [bq.reshape(HEADS, HD).T,
         (V_NUM / Q_NUM) * bv.reshape(HEADS, HD).T], axis=0)
    ball = np.concatenate([
        bv.reshape(IB, 128).T, bqw.reshape(IB, 128).T,
        fbias, bo.reshape(NB, 128).T], axis=1).astype(np.float32)
    ident = np.zeros((128, 8), np.float32)
    ident[:8, :8] = np.eye(8)
    head_cols = np.concatenate([
        wcore.astype(BF16), ident.astype(BF16),
        np.ascontiguousarray(ball).view(BF16)], axis=1)
    wrest_cols = wrest.astype(BF16)
    in_maps = []
    for i in range(N_CORES):
        bcols = []
        for bi in range(BPC):
            bidx = i * BPC + bi
            vt = v[bidx].T.reshape(DC, 128, V_NUM).transpose(1, 0, 2).reshape(128, DC * V_NUM)
            qt = q[bidx].T
            bcols.append(np.concatenate([vt.astype(BF16), qt.astype(BF16)], axis=1))
        packed = np.concatenate([head_cols] + bcols + [wrest_cols], axis=1)
        in_maps.append({"packed": np.ascontiguousarray(packed)})
    return in_maps


def kernel(**inputs):
    from concourse.bass_utils import run_bass_kernel_spmd

    nc = _get_nc()
    in_maps = _host_prep(**inputs)
    res = run_bass_kernel_spmd(nc, in_maps, core_ids=list(range(N_CORES)))
    out = np.empty((B, HIDDEN), np.float32)
    for i in range(N_CORES):
        out[i * BPC:(i + 1) * BPC] = np.asarray(res.results[i]["outT"]).T
    return out


# revision 32
# speedup vs baseline: 1.1112x; 1.0664x over previous
"""Trainium2 Bass kernel for a BAN (bilinear attention network) layer.

Reference computation (per batch b, head h, hd=64, scale=hd**-0.5):
    vp = (v @ Wv + bv)  -> [V=1024, 512] split into heads [h, V, 64]
    qp = (q @ Wq + bq)  -> [Q=512, 512]  split into heads [h, Q, 64]
    logits = vp_h @ att_w_h @ qp_h^T * scale        [V, Q]
    w = softmax(logits, axis=-1)
    pooled_v = mean_v(w @ qp_h)          [64]
    pooled_q = mean_q(w^T @ vp_h)        [64]
    fused = concat per head [pooled_v, pooled_q] -> [1024]
    out = relu(fused @ Wo + bo)          [512]

Key algebraic simplifications (validated vs ref):
  * rows of w sum to 1 => pooled_q = (1/Q) * colsum_v(vp_h)
  * pooled_v = z @ (q @ Wq)_h + bq_h with z = (1/V) sum_v e[v,:]/s[v],
    e = exp(logits), s = rowsum(e); z computed as a TensorE matmul with
    the scaled reciprocal rowsums (rb) as the stationary operand
  * att_w and the softmax scale are folded into Wq on the host

Performance structure (baseline 174us):
  * input DMA split in 4 (core weights / b0 acts / b1 acts / tail weights)
    so the first projection starts ~11us in instead of ~20us
  * exp runs on ScalarE over [128,1024] psum PAIR tiles (two v-chunks in
    adjacent psum banks, 3-deep rotation so the PE always has a free slot)
  * softmax rowsums: every engine reduces at ~1 elem/cycle, so the second
    pass over e is split between DVE (tensor_scalar+accum_out, 759ns/tile)
    and ACT (exp as two [128,512] ACTIVATEs with accum_out, +711ns/pair
    over a fused pair-exp) to balance the engines
  * z rows accumulate at psum partitions 0 (even head) / 64 (odd head) of
    one [128,512] bank; per-head drains keep the parity split and two
    restack DMAs stack them for the transpose
  * the two heads of a pair run their logits matmuls on disjoint 64-row
    groups of the PE (dual-issue); pooled_v/pooled_q matmuls process head
    PAIRS with a host-side row permutation of Wo absorbing the layout
  * projections share the pair-tile psum pool ([128,1024] groups, single
    wide drains); batch-0's ride ScalarE (idle before the first exp)

Sharding: data-parallel over batch, 2 batches per core, params replicated,
no collectives.
"""

import numpy as np
import ml_dtypes

BF16 = ml_dtypes.bfloat16

B, V_NUM, Q_NUM = 16, 1024, 512
V_DIM, Q_DIM = 256, 128
HIDDEN, HEADS, HD = 512, 8, 64
SCALE = HD ** -0.5

N_CORES = 8
BPC = B // N_CORES          # batches per core
DC = V_DIM // 128           # d-chunks of v (2)
IB = HIDDEN // 128          # i-blocks of hidden (4)
QC = Q_NUM // 128           # q-chunks (4)
VCH = V_NUM // 128          # v-chunks of 128 (8)
VPC = VCH // 2              # v-chunk pairs (4)
NB = HIDDEN // 128          # out feature blocks (4)
KC = (2 * HEADS * HD) // 128  # fused feature chunks of 128 (8)

# packed column layout (bf16 cols): core weights first so compute can start
# as soon as DMA1+DMA2 land. ball carries f32 data as raw bf16-pair bits
# (biases, fused bias cols, f32 identity for the transposes).
N_BALL_F32 = 2 * IB + HEADS + NB + 8
PCK_WCORE = 0                                  # wv (2*512) + wqw (512)
PCK_BALL = PCK_WCORE + (DC + 1) * HIDDEN
PCK_B0 = PCK_BALL + 2 * N_BALL_F32             # per-batch vt+qt
PCK_B1 = PCK_B0 + DC * V_NUM + Q_NUM
PCK_WREST = PCK_B1 + DC * V_NUM + Q_NUM        # wq (512) + wo (8*512)
PCK_COLS = PCK_WREST + (1 + KC) * HIDDEN

_CACHE = {}


def _build_nc():
    from contextlib import ExitStack

    import concourse.tile as tile
    from concourse import bacc, mybir
    from concourse.tile import add_dep_helper

    f32 = mybir.dt.float32
    bf16 = mybir.dt.bfloat16
    AF = mybir.ActivationFunctionType
    ALU = mybir.AluOpType

    nc = bacc.Bacc("TRN2", target_bir_lowering=False)

    packed_p = nc.declare_dram_parameter("packed", [128, PCK_COLS], bf16, isOutput=False)
    outT_p = nc.declare_dram_parameter("outT", [HIDDEN, BPC], f32, isOutput=True)

    with tile.TileContext(nc) as tc, ExitStack() as ctx:
        const = ctx.enter_context(tc.tile_pool(name="const", bufs=1))
        work = ctx.enter_context(tc.tile_pool(name="work", bufs=1))
        epool = ctx.enter_context(tc.tile_pool(name="epool", bufs=40))
        spool = ctx.enter_context(tc.tile_pool(name="spool", bufs=32))
        junk = ctx.enter_context(tc.tile_pool(name="junk", bufs=2))
        ps_pair = ctx.enter_context(tc.tile_pool(name="ps_pair", bufs=3, space="PSUM"))
        ps_z = ctx.enter_context(tc.tile_pool(name="ps_z", bufs=1, space="PSUM"))
        ps_sm = ctx.enter_context(tc.tile_pool(name="ps_sm", bufs=1, space="PSUM"))

        class SlotGuard:
            """Explicit WAR edges for psum slot reuse: the first writer of
            allocation i+bufs must wait for all readers of allocation i."""

            def __init__(self, bufs):
                self.bufs = bufs
                self.hist = []

            def alloc(self):
                self.hist.append([None, []])
                return len(self.hist) - 1

            def writer(self, idx, mi):
                if self.hist[idx][0] is None:
                    self.hist[idx][0] = mi
                    prev = idx - self.bufs
                    if prev >= 0:
                        for r in self.hist[prev][1]:
                            add_dep_helper(mi.ins, r.ins, sync=True,
                                           reason="psum slot WAR guard")
                return mi

            def reader(self, idx, mi):
                self.hist[idx][1].append(mi)
                return mi

        g_pair = SlotGuard(3)
        g_z = SlotGuard(1)
        g_sm = SlotGuard(1)

        # prewarm the ACT spline table (exp_and_others) while the input DMAs
        # are in flight, so the 1.3us ACT_TABLE_LOAD is off the critical path
        warm_sb = const.tile([1, 8], f32, tag="warm")
        nc.vector.memset(warm_sb[:], 0.0)
        nc.scalar.activation(warm_sb[0:1, 4:5], warm_sb[0:1, 0:1], AF.Exp)

        # ---- constants + inputs: 4 DMAs over one packed param ----
        packed_sb = const.tile([128, PCK_COLS], bf16, tag="packed")
        nc.sync.dma_start(packed_sb[:, PCK_WCORE:PCK_B0],
                          packed_p[:, PCK_WCORE:PCK_B0])
        nc.sync.dma_start(packed_sb[:, PCK_B0:PCK_B1], packed_p[:, PCK_B0:PCK_B1])
        nc.sync.dma_start(packed_sb[:, PCK_B1:PCK_WREST],
                          packed_p[:, PCK_B1:PCK_WREST])
        nc.sync.dma_start(packed_sb[:, PCK_WREST:PCK_COLS],
                          packed_p[:, PCK_WREST:PCK_COLS])

        wcore_sb = packed_sb[:, PCK_WCORE:PCK_BALL].rearrange(
            "p (w h) -> p w h", w=DC + 1)
        wv_sb = wcore_sb[:, 0:DC]
        wqw_sb = wcore_sb[:, DC]
        ball_sb = packed_sb[:, PCK_BALL:PCK_B0].bitcast(f32)
        bv_sb = ball_sb[:, 0:IB]
        bqw_sb = ball_sb[:, IB:2 * IB]
        fb_sb = ball_sb[:, 2 * IB:2 * IB + HEADS]
        bo_sb = ball_sb[:, 2 * IB + HEADS:2 * IB + HEADS + NB]
        identf_sb = ball_sb[0:8, 2 * IB + HEADS + NB:]
        bact_sb = [packed_sb[:, PCK_B0:PCK_B1], packed_sb[:, PCK_B1:PCK_WREST]]
        vt_sb = [a[:, :DC * V_NUM].rearrange("p (c v) -> p c v", c=DC)
                 for a in bact_sb]
        qt_sb = [a[:, DC * V_NUM:] for a in bact_sb]
        wrest_sb = packed_sb[:, PCK_WREST:PCK_COLS].rearrange(
            "p (w h) -> p w h", w=1 + KC)
        wq_sb = wrest_sb[:, 0]
        wo_sb = wrest_sb[:, 1:]

        # ---- long-lived activations ----
        vpT_sb = work.tile([128, BPC, IB, V_NUM], bf16, tag="vpt")
        qpwT_sb = work.tile([128, BPC, IB, Q_NUM], bf16, tag="qpwt")
        qp_sb = work.tile([128, BPC, QC, HIDDEN], bf16, tag="qp")
        # z rows land on psum partitions 0 (even head) / 64 (odd head); the
        # drains keep that split and two restack DMAs stack them onto
        # partitions 0-7 in parity order: row r<4 = head 2r, r>=4 = head
        # 2(r-4)+1, i.e. zT column t<4 = head 2t, column 4+t = head 2t+1.
        zrow_sb = work.tile([128, BPC, HEADS // 2, Q_NUM], f32, tag="zrow")
        zstack_sb = work.tile([8, BPC, Q_NUM], f32, tag="zstack")
        zT_sb = work.tile([128, BPC, QC, HEADS], bf16, tag="zT")
        fusedT_sb = work.tile([128, KC, BPC], bf16, tag="fused")
        outT_sb = work.tile([128, NB, BPC], f32, tag="outT")
        cv_sb = work.tile([128, BPC, DC], f32, tag="cv")
        cvb_sb = work.tile([128, BPC, DC], bf16, tag="cvb")

        # softmax rowsum routing: (t, pc) pair-slots in ACT_ACCUM_PC run the
        # exp as two [128,512] ACTIVATEs with accum_out (ScalarE), the rest
        # as one [128,1024] pair-exp with the rowsums on DVE.
        ACT_ACCUM_PC = {(0, 1), (0, 3), (1, 1), (2, 1), (2, 3), (3, 1)}

        def s_reduce(e2_t, s_t, pc, b):
            for i in range(2):
                c = 2 * pc + i
                half = e2_t[:, i * Q_NUM:(i + 1) * Q_NUM]
                jt = junk.tile([128, Q_NUM], bf16, tag="junk")
                nc.vector.tensor_scalar(
                    jt[:], half, 1.0, None, ALU.mult, ALU.add,
                    accum_out=s_t[:, c:c + 1])

        # ---- projections: [128,1024] groups on the shared pair-tile pool ----
        def vpt_pair(b, ib, act):
            ps = ps_pair.tile([128, 2 * Q_NUM], f32, tag="pair")
            gi = g_pair.alloc()
            for vb in range(2):
                for dc in range(DC):
                    g_pair.writer(gi, nc.tensor.matmul(
                        ps[:, vb * 512:(vb + 1) * 512],
                        lhsT=wv_sb[:, dc, ib * 128:(ib + 1) * 128],
                        rhs=vt_sb[b][:, dc, vb * 512:(vb + 1) * 512],
                        start=(dc == 0), stop=(dc == DC - 1)))
            if act:
                g_pair.reader(gi, nc.scalar.activation(
                    vpT_sb[:, b, ib, :], ps[:], AF.Identity,
                    bias=bv_sb[:, ib:ib + 1]))
            else:
                g_pair.reader(gi, nc.vector.tensor_scalar_add(
                    vpT_sb[:, b, ib, :], ps[:], bv_sb[:, ib:ib + 1]))

        def qpwt_pair(b, ibs, act):
            ps = ps_pair.tile([128, 2 * Q_NUM], f32, tag="pair")
            gi = g_pair.alloc()
            for k, ib in enumerate(ibs):
                g_pair.writer(gi, nc.tensor.matmul(
                    ps[:, k * 512:(k + 1) * 512],
                    lhsT=wqw_sb[:, ib * 128:(ib + 1) * 128],
                    rhs=qt_sb[b][:], start=True, stop=True))
            for k, ib in enumerate(ibs):
                if act:
                    g_pair.reader(gi, nc.scalar.activation(
                        qpwT_sb[:, b, ib, :], ps[:, k * 512:(k + 1) * 512],
                        AF.Identity, bias=bqw_sb[:, ib:ib + 1]))
                else:
                    g_pair.reader(gi, nc.vector.tensor_scalar_add(
                        qpwT_sb[:, b, ib, :], ps[:, k * 512:(k + 1) * 512],
                        bqw_sb[:, ib:ib + 1]))

        def qp_pair(b, qc0, act):
            ps = ps_pair.tile([128, 2 * Q_NUM], f32, tag="pair")
            gi = g_pair.alloc()
            for k in range(2):
                g_pair.writer(gi, nc.tensor.matmul(
                    ps[:, k * 512:(k + 1) * 512],
                    lhsT=qt_sb[b][:, (qc0 + k) * 128:(qc0 + k + 1) * 128],
                    rhs=wq_sb[:], start=True, stop=True))
            dest = qp_sb[:, b, qc0:qc0 + 2].rearrange("p q h -> p (q h)")
            if act:
                g_pair.reader(gi, nc.scalar.activation(dest, ps[:], AF.Copy))
            else:
                g_pair.reader(gi, nc.vector.tensor_copy(dest, ps[:]))

        def prologue_thunks(b, skip_ib0=False):
            """Projection/pooled-q thunks spread through the main loop."""
            thunks = []

            def th(fn):
                thunks.append(fn)

            for ib in range(1 if skip_ib0 else 0, IB):
                th(lambda ib=ib: vpt_pair(b, ib, act=False))
            th(lambda: qpwt_pair(b, [1, 2] if skip_ib0 else [0, 1], act=False))
            th(lambda: qpwt_pair(b, [3] if skip_ib0 else [2, 3], act=False))
            if b == 0:
                th(lambda: qp_pair(b, 0, act=False))
                th(lambda: qp_pair(b, 2, act=False))

            def cv_group():
                for dc in range(DC):
                    jt = junk.tile([128, V_NUM], bf16, tag="junkb")
                    nc.vector.tensor_scalar(
                        jt[:], vt_sb[b][:, dc, :], 1.0, None, ALU.mult,
                        ALU.add, accum_out=cv_sb[:, b, dc:dc + 1])
                    nc.gpsimd.tensor_copy(
                        cvb_sb[:, b, dc:dc + 1], cv_sb[:, b, dc:dc + 1])
            th(cv_group)
            # pooled_q head-pairs: lhsT spans a full ib block (2 heads), out
            # [128,1] = [pq_2ib | pq_2ib+1] -> fusedT column 4+ib
            for ib in range(IB):
                def pq_group(ib=ib):
                    psq = ps_sm.tile([128, 8], f32, tag="sm")
                    gi = g_sm.alloc()
                    for dc in range(DC):
                        g_sm.writer(gi, nc.tensor.matmul(
                            psq[:, 0:1],
                            lhsT=wv_sb[:, dc, ib * 128:(ib + 1) * 128],
                            rhs=cvb_sb[:, b, dc:dc + 1],
                            start=(dc == 0), stop=(dc == DC - 1)))
                    g_sm.reader(gi, nc.vector.tensor_scalar(
                        fusedT_sb[:, 4 + ib, b:b+1], psq[:, 0:1],
                        1.0 / Q_NUM, fb_sb[:, 4 + ib:5 + ib],
                        ALU.mult, ALU.add))
                th(pq_group)
            return thunks

        # ---- z tail: restack z rows, transpose, pooled_v head-pairs ----
        def ztail_thunks(b):
            thunks = []

            def th(fn):
                thunks.append(fn)

            if b == BPC - 1:
                th(lambda: qp_pair(b, 0, act=True))
                th(lambda: qp_pair(b, 2, act=True))

            def restack():
                nc.sync.dma_start(
                    zstack_sb[0:4, b, :],
                    zrow_sb[0:1, b].rearrange("p t q -> p (t q)"))
                nc.sync.dma_start(
                    zstack_sb[4:8, b, :],
                    zrow_sb[64:65, b].rearrange("p t q -> p (t q)"))
            th(restack)
            for qc in range(QC):
                def tr_group(qc=qc):
                    pst = ps_sm.tile([128, 8], f32, tag="sm")
                    gi = g_sm.alloc()
                    g_sm.writer(gi, nc.tensor.transpose(
                        pst[:], zstack_sb[:, b, qc * 128:(qc + 1) * 128],
                        identf_sb[:]))
                    g_sm.reader(gi, nc.vector.tensor_copy(
                        zT_sb[:, b, qc, :], pst[:]))
                th(tr_group)
            # pooled_v head-pairs: lhsT = qp block of heads (2t, 2t+1), rhs =
            # zT columns (t, 4+t) via a rearranged strided view; out [128,2]
            # holds pv_2t in rows 0:64 of col 0, pv_2t+1 in rows 64:128 of
            # col 1 -> fusedT column t
            for t in range(HEADS // 2):
                def pv_group(t=t):
                    psv = ps_sm.tile([128, 8], f32, tag="sm")
                    gi = g_sm.alloc()
                    for qc in range(QC):
                        zview = zT_sb[:, b, qc].rearrange(
                            "p (par t) -> p t par", par=2)
                        g_sm.writer(gi, nc.tensor.matmul(
                            psv[:, 0:2],
                            lhsT=qp_sb[:, b, qc, 2 * t * 64:(2 * t + 2) * 64],
                            rhs=zview[:, t, :],
                            start=(qc == 0), stop=(qc == QC - 1)))
                    drain = (nc.scalar.activation if b == BPC - 1
                             else None)
                    if drain is not None:
                        g_sm.reader(gi, nc.scalar.activation(
                            fusedT_sb[0:64, t, b:b+1], psv[0:64, 0:1],
                            AF.Identity, bias=fb_sb[0:64, t:t + 1]))
                        g_sm.reader(gi, nc.scalar.activation(
                            fusedT_sb[64:128, t, b:b+1], psv[64:128, 1:2],
                            AF.Identity, bias=fb_sb[64:128, t:t + 1]))
                    else:
                        g_sm.reader(gi, nc.vector.tensor_scalar(
                            fusedT_sb[0:64, t, b:b+1], psv[0:64, 0:1],
                            1.0, fb_sb[0:64, t:t + 1], ALU.mult, ALU.add))
                        g_sm.reader(gi, nc.vector.tensor_scalar(
                            fusedT_sb[64:128, t, b:b+1], psv[64:128, 1:2],
                            1.0, fb_sb[64:128, t:t + 1], ALU.mult, ALU.add))
                th(pv_group)
            return thunks

        # ---- main loop ----
        def emit_main(b, pre_work):
            pending = None

            def colsum(pend, pc):
                t, zps, zgi, psides = pend
                for h, e_list, rb_t in psides:
                    hb = 64 * (h % 2)
                    for i in range(2):
                        c = 2 * pc + i
                        g_z.writer(zgi, nc.tensor.matmul(
                            zps[hb:hb + 1, :],
                            lhsT=rb_t[:, c:c + 1],
                            rhs=e_list[pc][:, i * Q_NUM:(i + 1) * Q_NUM],
                            start=(c == 0), stop=(c == VCH - 1)))
                if pc == VPC - 1:
                    for h, e_list, rb_t in psides:
                        hb = 64 * (h % 2)
                        if b == BPC - 1 and t == HEADS // 2 - 1:
                            # after the final exp: ACT is idle
                            g_z.reader(zgi, nc.scalar.activation(
                                zrow_sb[hb:hb + 1, b, t, :],
                                zps[hb:hb + 1, :], AF.Copy))
                        else:
                            g_z.reader(zgi, nc.vector.tensor_copy(
                                zrow_sb[hb:hb + 1, b, t, :], zps[hb:hb + 1, :]))

            for t in range(HEADS // 2):
                sides = []
                for h in (2 * t, 2 * t + 1):
                    s_t = spool.tile([128, VCH], f32, tag="s")
                    sides.append([h, 64 * (h % 2), s_t, []])
                for pc in range(VPC):
                    for side in sides:
                        h, hb, s_t, e_list = side
                        ps = ps_pair.tile([128, 2 * Q_NUM], f32, tag="pair")
                        gi = g_pair.alloc()
                        for i in range(2):
                            c = 2 * pc + i
                            g_pair.writer(gi, nc.tensor.matmul(
                                ps[:, i * Q_NUM:(i + 1) * Q_NUM],
                                lhsT=vpT_sb[hb:hb + 64, b, t, c * 128:(c + 1) * 128],
                                rhs=qpwT_sb[hb:hb + 64, b, t, :],
                                start=True, stop=True))
                        e2_t = epool.tile([128, 2 * Q_NUM], bf16, tag="e")
                        if (t, pc) in ACT_ACCUM_PC:
                            for i in range(2):
                                c = 2 * pc + i
                                g_pair.reader(gi, nc.scalar.activation(
                                    e2_t[:, i * Q_NUM:(i + 1) * Q_NUM],
                                    ps[:, i * Q_NUM:(i + 1) * Q_NUM], AF.Exp,
                                    accum_out=s_t[:, c:c + 1]))
                        else:
                            g_pair.reader(gi, nc.scalar.activation(
                                e2_t[:], ps[:], AF.Exp))
                            s_reduce(e2_t, s_t, pc, b)
                        e_list.append(e2_t)
                    if pending is not None:
                        colsum(pending, pc)
                    for _ in range(2):
                        if pre_work:
                            pre_work.pop(0)()
                # reciprocals for this pair (DVE, overlaps next pair)
                zps = ps_z.tile([128, Q_NUM], f32, tag="z")
                zgi = g_z.alloc()
                new_sides = []
                for h, hb, s_t, e_list in sides:
                    r_t = spool.tile([128, VCH], f32, tag="r")
                    nc.vector.reciprocal(r_t[:], s_t[:])
                    rb_t = spool.tile([128, VCH], bf16, tag="rb")
                    nc.vector.tensor_scalar_mul(
                        rb_t[:], r_t[:], 1.0 / V_NUM)
                    new_sides.append((h, e_list, rb_t))
                pending = (t, zps, zgi, new_sides)
            # last pair's colsum (trailing, overlaps the next stream)
            for pc in range(VPC):
                colsum(pending, pc)
                if pre_work:
                    pre_work.pop(0)()
            while pre_work:
                pre_work.pop(0)()

        vpt_pair(0, 0, act=True)
        qpwt_pair(0, [0], act=True)
        pro0 = prologue_thunks(0, skip_ib0=True)
        emit_main(0, pro0 + prologue_thunks(1))
        emit_main(1, ztail_thunks(0))
        for fn in ztail_thunks(1):
            fn()

        # ---- epilogue: out = relu(fused @ Wo + bo), computed transposed ----
        for nb in range(NB):
            pso = ps_sm.tile([128, 8], f32, tag="sm")
            gi = g_sm.alloc()
            for kc in range(KC):
                g_sm.writer(gi, nc.tensor.matmul(
                    pso[:, 0:BPC],
                    lhsT=wo_sb[:, kc, nb * 128:(nb + 1) * 128],
                    rhs=fusedT_sb[:, kc, :],
                    start=(kc == 0), stop=(kc == KC - 1)))
            g_sm.reader(gi, nc.scalar.activation(
                outT_sb[:, nb, :], pso[:, 0:BPC], AF.Relu,
                bias=bo_sb[:, nb:nb + 1]))
        nc.sync.dma_start(
            outT_p[:].rearrange("(o p) b -> p o b", p=128), outT_sb[:])

    nc.compile()
    return nc


def _get_nc():
    if "nc" not in _CACHE:
        _CACHE["nc"] = _build_nc()
    return _CACHE["nc"]


def _fused_perm():
    """Row permutation of Wo matching the on-device fused layout:
    fusedT column t<4 = [pv_2t | pv_2t+1], column 4+ib = [pq_2ib | pq_2ib+1].
    Returns perm with packed_row[kc*128+p] = orig_row[perm[kc*128+p]]."""
    perm = np.empty(2 * HEADS * HD, np.int64)
    for kc in range(KC):
        for p in range(128):
            if kc < 4:
                h = 2 * kc + (1 if p >= 64 else 0)
                r = h * 128 + (p % 64)
            else:
                ib = kc - 4
                h = 2 * ib + (1 if p >= 64 else 0)
                r = h * 128 + 64 + (p % 64)
            perm[kc * 128 + p] = r
    return perm


def _host_prep(v, q, Wv, bv, Wq, bq, att_w, Wo, bo):
    """Host-side layout transforms + weight folding. Returns per-core in_maps."""
    v = np.asarray(v, np.float32)
    q = np.asarray(q, np.float32)
    Wv = np.asarray(Wv, np.float32)
    bv = np.asarray(bv, np.float32)
    Wq = np.asarray(Wq, np.float32)
    bq = np.asarray(bq, np.float32)
    att_w = np.asarray(att_w, np.float32)
    Wo = np.asarray(Wo, np.float32)
    bo = np.asarray(bo, np.float32)

    # fold att_w and softmax scale into the q projection
    Wq_h = Wq.reshape(Q_DIM, HEADS, HD)
    Wqw = (SCALE * np.einsum("dhj,hij->dhi", Wq_h, att_w)).reshape(Q_DIM, HIDDEN)
    bqw = (SCALE * np.einsum("hj,hij->hi", bq.reshape(HEADS, HD), att_w)).reshape(HIDDEN)

    wcore = np.concatenate([
        Wv.reshape(DC, 128, HIDDEN).transpose(1, 0, 2),
        Wqw.reshape(1, 128, HIDDEN).transpose(1, 0, 2),
    ], axis=1).reshape(128, (DC + 1) * HIDDEN)
    perm = _fused_perm()
    WoP = Wo[perm]
    wrest = np.concatenate([
        Wq.reshape(1, 128, HIDDEN).transpose(1, 0, 2),
        WoP.reshape(KC, 128, HIDDEN).transpose(1, 0, 2),
    ], axis=1).reshape(128, (1 + KC) * HIDDEN)
    # fused bias columns in the permuted layout: col t<4 = [bq_2t | bq_2t+1]
    # (pv bias), col 4+ib = (V/Q)*[bv_2ib | bv_2ib+1] (pq bias)
    bq_h = bq.reshape(HEADS, HD)
    bv_h = bv.reshape(HEADS, HD)
    fbias = np.empty((128, HEADS), np.float32)
    for t in range(4):
        fbias[0:64, t] = bq_h[2 * t]
        fbias[64:128, t] = bq_h[2 * t + 1]
    for ib in range(4):
        fbias[0:64, 4 + ib] = (V_NUM / Q_NUM) * bv_h[2 * ib]
        fbias[64:128, 4 + ib] = (V_NUM / Q_NUM) * bv_h[2 * ib + 1]
    identf = np.zeros((128, 8), np.float32)
    identf[:8, :8] = np.eye(8)
    ball = np.concatenate([
        bv.reshape(IB, 128).T, bqw.reshape(IB, 128).T,
        fbias, bo.reshape(NB, 128).T, identf], axis=1).astype(np.float32)
    head_cols = np.concatenate([
        wcore.astype(BF16),
        np.ascontiguousarray(ball).view(BF16)], axis=1)
    wrest_cols = wrest.astype(BF16)
    in_maps = []
    for i in range(N_CORES):
        bcols = []
        for bi in range(BPC):
            bidx = i * BPC + bi
            vt = v[bidx].T.reshape(DC, 128, V_NUM).transpose(1, 0, 2).reshape(128, DC * V_NUM)
            qt = q[bidx].T
            bcols.append(np.concatenate([vt.astype(BF16), qt.astype(BF16)], axis=1))
        packed = np.concatenate([head_cols] + bcols + [wrest_cols], axis=1)
        in_maps.append({"packed": np.ascontiguousarray(packed)})
    return in_maps


def kernel(**inputs):
    from concourse.bass_utils import run_bass_kernel_spmd

    nc = _get_nc()
    in_maps = _host_prep(**inputs)
    res = run_bass_kernel_spmd(nc, in_maps, core_ids=list(range(N_CORES)))
    out = np.empty((B, HIDDEN), np.float32)
    for i in range(N_CORES):
        out[i * BPC:(i + 1) * BPC] = np.asarray(res.results[i]["outT"]).T
    return out
